# revision 28
# baseline (speedup 1.0000x reference)
"""Trainium2 Bass kernel for nn_Net_91113436217372.

Dense CNN: 13x (3->3ch 3x3 conv) + 5 maxpools on a 1x3x5120x5120 image,
then fc1 [1024, 76800] and fc2 [1024, 1024] (both linear, no bias).

Strategy (8 NeuronCores, fully independent SPMD -- no collectives):
  - Shard H into 8 bands with redundant halo compute (each core gets the
    input rows it needs for its 20 final rows: 820 rows incl. halo).
  - Convs as banded-weight matmuls: stationary B_dx[(ci,y_in)->(co,y_out)]
    encodes all (ci,dy) taps; 3 PSUM-accumulated passes over dx (free-dim
    shifts of the rhs image tile). K=126, M=128, N>=256 float32r.
  - Maxpool: y-pairs via M-ordering (ph at partitions 0-59/64-123) +
    tensor_max; x-pairs via strided tensor_max on the free dim.
  - Conv path float32r (tf32-class, full PE rate at N>=256), fp32 PSUM.
  - fc1/fc2 are linear with no nonlinearity between, so each core pushes
    its partial fc1 sum through fc2 (bf16 weights) and the host sums the
    8 core outputs.
"""
import sys
import numpy as np

for p in ("/opt/trn_rl_repo",):
    if p not in sys.path:
        sys.path.insert(0, p)

import ml_dtypes
import concourse.bass as bass
import concourse.bacc as bacc
import concourse.tile as tile
import concourse.mybir as mybir
from concourse import bass_utils
from contextlib import ExitStack

BF16 = mybir.dt.bfloat16
F32 = mybir.dt.float32
F32R = mybir.dt.float32r
NPBF16 = ml_dtypes.bfloat16

N_CORES = 8
H = W0 = 5120
BAND = 820          # input rows per core (640 + 90 halo each side)
BAND_OFF = -90      # core c's band starts at global row 640*c - 90

# Per-block geometry: (n_convs, R_in_rows, width)
BLOCK_DEFS = [(2, 820, 5120), (2, 408, 2560), (3, 202, 1280), (3, 98, 640), (3, 46, 320)]


def _layer_table():
    layers = []
    l = 0
    in_name = "x"
    for b, (n_convs, R, W) in enumerate(BLOCK_DEFS):
        for j in range(1, n_convs + 1):
            pool = j == n_convs
            if pool and b == len(BLOCK_DEFS) - 1:
                out_name = "feat"
            else:
                out_name = f"s{l}"
            layers.append(dict(l=l, in_name=in_name, out_name=out_name,
                               R=R, W=W, j=j, pool=pool))
            in_name = out_name
            l += 1
    return layers

LAYERS = _layer_table()


def _tiles_for(layer):
    # out-row range of layer (local coords): [j, R-j); tiles of 40 rows
    j, R = layer["j"], layer["R"]
    start, end = j, R - j
    bases = list(range(start, end - 40 + 1, 40))
    if not bases:
        bases = [start]
    last = end - 40
    if bases[-1] != last:
        bases.append(last)  # shifted-up partial tile (even shift keeps parity)
    return bases


def _x_subtiles(W):
    # chunks of <=512, all >=256 (float32r full-rate needs N>=256), even
    subs = []
    c = 0
    while c < W:
        rem = W - c
        if rem <= 512:
            nn = rem
        elif rem < 768:
            nn = (rem // 2 + 1) & ~1
        else:
            nn = 512
        subs.append((c, nn))
        c += nn
    return subs


Z_TOP = [90, 44, 21, 9, 3]
Z_BOT = [730, 364, 181, 89, 43]
BLK_W = [5120, 2560, 1280, 640, 320]


def _mask_cols():
    """Strips (layer l, base) that contain an image-boundary bleed row.
    Returns ordered list of (l, base, pool, entries) where entries =
    [(partition, which)] with which 0=top(core0) 1=bottom(core7)."""
    cols = []
    for layer in LAYERS:
        l, j, Wd, pool = layer["l"], layer["j"], layer["W"], layer["pool"]
        blk = BLK_W.index(Wd)
        for base in _tiles_for(layer):
            entries = []
            for (rr, which) in ((Z_TOP[blk] - 1, 0), (Z_BOT[blk], 1)):
                if base <= rr < base + 40:
                    for co in range(3):
                        if pool:
                            entries.append((co * 20 + (rr - base) // 2, which))
                        else:
                            entries.append((co * 40 + (rr - base), which))
            if entries:
                cols.append((l, base, pool, entries))
    return cols

MASK_COLS = _mask_cols()
N_MASK = len(MASK_COLS)


def build_program(dbg=False, n_layers=13, do_fc=True):
    nc = bacc.Bacc("TRN2", target_bir_lowering=False, debug=False,
                   num_devices=N_CORES)
    dbg_kind = dict(kind="ExternalOutput") if dbg else {}

    x_t = nc.dram_tensor("x", [3, BAND, W0 + 2], F32R, kind="ExternalInput").ap()
    b_ts = {}
    for layer in LAYERS:
        l = layer["l"]
        for dx in range(3):
            b_ts[(l, dx)] = nc.dram_tensor(f"b{l}_{dx}", [126, 128], F32R,
                                           kind="ExternalInput").ap()
    mask_t = nc.dram_tensor("mask", [128, max(N_MASK, 1)], F32R,
                            kind="ExternalInput").ap()
    w1t_t = nc.dram_tensor("w1t", [9600, 1024], BF16, kind="ExternalInput").ap()
    w2t_t = nc.dram_tensor("w2t", [1024, 1024], BF16, kind="ExternalInput").ap()
    q_t = nc.dram_tensor("q", [1, 1024], F32, kind="ExternalOutput").ap()

    spills = {"x": x_t}
    for layer in LAYERS[:-1]:
        out = layer["out_name"]
        if layer["pool"]:
            bnext = [bd for bd in BLOCK_DEFS if bd[2] == layer["W"] // 2]
            Rn, Wn = bnext[0][1], bnext[0][2]
            spills[out] = nc.dram_tensor(out, [3, Rn, Wn + 2], F32R, **dbg_kind).ap()
        else:
            spills[out] = nc.dram_tensor(out, [3, layer["R"], layer["W"] + 2], F32R,
                                         **dbg_kind).ap()
    feat_t = nc.dram_tensor("feat", [9600], F32R, **dbg_kind).ap()
    spills["feat"] = feat_t

    with tile.TileContext(nc) as tc, ExitStack() as ctx:
        b_pool = ctx.enter_context(tc.tile_pool(name="bp", bufs=1))
        rhs_pool = ctx.enter_context(tc.tile_pool(name="rp", bufs=3))
        stg_pool = ctx.enter_context(tc.tile_pool(name="sp", bufs=2))
        pxy_pool = ctx.enter_context(tc.tile_pool(name="px", bufs=4))
        psum_pool = ctx.enter_context(tc.tile_pool(name="pp", bufs=6, space="PSUM"))
        fcp_pool = ctx.enter_context(tc.tile_pool(name="fp", bufs=1, space="PSUM"))
        w_pool = ctx.enter_context(tc.tile_pool(name="wp", bufs=3))
        misc_pool = ctx.enter_context(tc.tile_pool(name="mp", bufs=1))

        mask_sb = misc_pool.tile([128, max(N_MASK, 1)], F32R, tag="mask")
        nc.sync.dma_start(mask_sb[:], mask_t[:])
        mask_idx = {(l, base): i for i, (l, base, _, _) in enumerate(MASK_COLS)}

        # load all B matrices once
        b_sb = {}
        for layer in LAYERS[:n_layers]:
            l = layer["l"]
            for dx in range(3):
                t = b_pool.tile([126, 128], F32R, tag=f"B{l}_{dx}", name=f"B{l}_{dx}")
                nc.sync.dma_start(t[:], b_ts[(l, dx)][:])
                b_sb[(l, dx)] = t

        # one-time zero of spill halo columns (x is host-padded already)
        ztile = misc_pool.tile([128, 16], F32, tag="ztile")
        nc.vector.memset(ztile[:], 0.0)

        def _zsrc(R):
            for p in range(128, 0, -1):
                if R % p == 0 and R // p <= 16:
                    return ztile[0:p, 0:R // p].bitcast(F32R)
            raise ValueError(R)

        for layer in LAYERS[:n_layers]:
            if layer["out_name"] == "feat":
                continue
            sp_ap = spills[layer["out_name"]]
            Rsp, Wsp = sp_ap.shape[1], sp_ap.shape[2]
            for ci in range(3):
                nc.sync.dma_start(sp_ap[ci, :, 0:1], _zsrc(Rsp))
                nc.sync.dma_start(sp_ap[ci, :, Wsp - 1:Wsp], _zsrc(Rsp))

        # --- conv stack ---
        for layer in LAYERS[:n_layers]:
            l, j, Wd, pool = layer["l"], layer["j"], layer["W"], layer["pool"]
            in_ap = spills[layer["in_name"]]
            out_ap = spills[layer["out_name"]]
            subs = _x_subtiles(Wd)
            for base in _tiles_for(layer):
                rhs = rhs_pool.tile([126, Wd + 2], F32R, tag="rhs", name="rhs")
                for ci in range(3):
                    nc.sync.dma_start(rhs[ci * 42:(ci + 1) * 42, :],
                                      in_ap[ci, base - 1: base + 41, :])

                if pool:
                    pooled = stg_pool.tile([64, Wd // 2], F32R, tag="pooled",
                                           name="pooled")
                else:
                    stg = stg_pool.tile([120, Wd], F32R, tag="stg", name="stg")

                GRP = 6
                for g0 in range(0, len(subs), GRP):
                    grp = subs[g0:g0 + GRP]
                    pss = [psum_pool.tile([128, 512], F32, tag="cv", name="cv")
                           for _ in grp]
                    for dx in range(3):
                        for ps, (xs0, nn) in zip(pss, grp):
                            nc.tensor.matmul(
                                ps[:, :nn], b_sb[(l, dx)][:],
                                rhs[:, xs0 + dx: xs0 + dx + nn],
                                start=(dx == 0), stop=(dx == 2),
                                skip_group_check=True)
                    for ps, (xs0, nn) in zip(pss, grp):
                        if pool:
                            sl = slice(xs0 // 2, (xs0 + nn) // 2)
                            phi = pxy_pool.tile([64, 512], F32R, tag="phi", name="phi")
                            pym = pxy_pool.tile([64, 512], F32R, tag="pym", name="pym")
                            nc.vector.tensor_copy(phi[:, :nn], ps[64:128, :nn])
                            nc.vector.tensor_max(pym[:, :nn],
                                                 ps[0:64, :nn], phi[:, :nn])
                            nc.vector.tensor_max(pooled[:, sl],
                                                 pym[:, 0:nn:2], pym[:, 1:nn:2])
                        else:
                            nc.vector.tensor_copy(stg[:, xs0:xs0 + nn], ps[0:120, :nn])

                # zero the boundary-bleed rows (mask col is 0 only at the
                # bleed partitions on edge cores; all-ones elsewhere)
                mi = mask_idx.get((l, base))
                if mi is not None:
                    if pool:
                        nc.vector.tensor_scalar_mul(
                            pooled[0:64, :], pooled[0:64, :],
                            mask_sb[0:64, mi:mi + 1].bitcast(F32))
                    else:
                        nc.vector.tensor_scalar_mul(
                            stg[0:120, :], stg[0:120, :],
                            mask_sb[0:120, mi:mi + 1].bitcast(F32))

                if pool:
                    if layer["out_name"] == "feat":
                        nc.sync.dma_start(
                            feat_t.rearrange("(p f) -> p f", p=60), pooled[0:60, :])
                    else:
                        pbase = (base - j) // 2
                        for co in range(3):
                            nc.sync.dma_start(
                                out_ap[co, pbase: pbase + 20, 1: Wd // 2 + 1],
                                pooled[co * 20:(co + 1) * 20, :])
                else:
                    for co in range(3):
                        nc.sync.dma_start(
                            out_ap[co, base: base + 40, 1: Wd + 1],
                            stg[co * 40:(co + 1) * 40, :])

        if do_fc:
            # --- fc1: partial p = A_c @ W1_c.T (K=9600 contraction) ---
            a75f = misc_pool.tile([128, 75], F32R, tag="a75f")
            nc.sync.dma_start(a75f[:], feat_t.rearrange("(k p) -> p k", p=128))
            a75 = misc_pool.tile([128, 75], BF16, tag="a75")
            nc.vector.tensor_copy(a75[:], a75f[:])
            p0 = fcp_pool.tile([1, 512], F32, tag="fc0", name="p0")
            p1 = fcp_pool.tile([1, 512], F32, tag="fc1", name="p1")
            for k in range(75):
                wt = w_pool.tile([128, 1024], BF16, tag="w1t", name="w1t")
                nc.sync.dma_start(wt[:], w1t_t[k * 128:(k + 1) * 128, :])
                nc.tensor.matmul(p0[:], a75[:, k:k + 1], wt[:, 0:512],
                                 start=(k == 0), stop=(k == 74), skip_group_check=True)
                nc.tensor.matmul(p1[:], a75[:, k:k + 1], wt[:, 512:1024],
                                 start=(k == 0), stop=(k == 74), skip_group_check=True)
            p_sb = misc_pool.tile([1, 1024], BF16, tag="psb")
            nc.vector.tensor_copy(p_sb[:, 0:512], p0[:])
            nc.vector.tensor_copy(p_sb[:, 512:1024], p1[:])

            if dbg:
                pdbg_t = nc.dram_tensor("pdbg", [1, 1024], BF16,
                                        kind="ExternalOutput").ap()
                nc.sync.dma_start(pdbg_t[:], p_sb[:])

            # reshape p [1,1024] -> [128, 8] via DRAM bounce
            pflat_t = nc.dram_tensor("pflat", [1024], BF16).ap()
            nc.sync.dma_start(pflat_t.rearrange("(a f) -> a f", a=1), p_sb[:])
            p128 = misc_pool.tile([128, 8], BF16, tag="p128")
            nc.sync.dma_start(p128[:], pflat_t.rearrange("(k p) -> p k", p=128))

            # --- fc2: q = W2 @ p ---
            q0 = fcp_pool.tile([1, 512], F32, tag="fc0", name="q0")
            q1 = fcp_pool.tile([1, 512], F32, tag="fc1", name="q1")
            for k in range(8):
                wt2 = w_pool.tile([128, 1024], BF16, tag="w2t", name="w2t")
                nc.sync.dma_start(wt2[:], w2t_t[k * 128:(k + 1) * 128, :])
                nc.tensor.matmul(q0[:], p128[:, k:k + 1], wt2[:, 0:512],
                                 start=(k == 0), stop=(k == 7), skip_group_check=True)
                nc.tensor.matmul(q1[:], p128[:, k:k + 1], wt2[:, 512:1024],
                                 start=(k == 0), stop=(k == 7), skip_group_check=True)
            q_sb = misc_pool.tile([1, 1024], F32, tag="qsb")
            nc.vector.tensor_copy(q_sb[:, 0:512], q0[:])
            nc.vector.tensor_copy(q_sb[:, 512:1024], q1[:])
            nc.sync.dma_start(q_t[:], q_sb[:])
        else:
            dummy = misc_pool.tile([1, 1024], F32, tag="dummy")
            nc.vector.memset(dummy[:], 0.0)
            nc.sync.dma_start(q_t[:], dummy[:])

    nc.compile()
    return nc


# ---------------- host-side input prep ----------------

def _conv_Bs(w, pool):
    """w [co,ci,dy,dx] f32 -> 3 banded matrices [126,128] f32 (one per dx).

    M (out-partition) mapping:
      non-pool: m = co*40 + t           (t = row-in-tile, 0..39; cols 120+ zero)
      pool:     m = ph*64 + co*20 + y2  (t = 2*y2+ph; cols 60-63, 124+ zero)
    """
    m = np.arange(128)
    if pool:
        ph, rem = m // 64, m % 64
        co, y2 = rem // 20, rem % 20
        t = 2 * y2 + ph
        mvalid = rem < 60
    else:
        co, t = m // 40, m % 40
        mvalid = m < 120
    r = np.arange(42)
    dy = r[:, None] - t[None, :]              # [42, 128]
    valid = (dy >= 0) & (dy <= 2) & mvalid[None, :]
    dyc = np.clip(dy, 0, 2)
    co2 = np.broadcast_to(np.clip(co, 0, 2)[None, :], (42, 128))
    Bs = []
    for dx in range(3):
        B = np.zeros((126, 128), np.float32)
        for ci in range(3):
            vals = w[co2, ci, dyc, dx]
            B[ci * 42:(ci + 1) * 42, :] = np.where(valid, vals, 0.0)
        Bs.append(B)
    return Bs


def _prep_in_maps(x, ws, fc1_w, fc2_w):
    x = np.asarray(x)[0]                      # [3, H, W]
    xb = np.asarray(x, np.float32)
    common = {}
    for layer in LAYERS:
        l = layer["l"]
        Bs = _conv_Bs(np.asarray(ws[l], np.float32), layer["pool"])
        for dx in range(3):
            common[f"b{l}_{dx}"] = Bs[dx]
    common["w2t"] = np.ascontiguousarray(np.asarray(fc2_w, np.float32).T).astype(NPBF16)

    fc1_w = np.asarray(fc1_w, np.float32)
    in_maps = []
    for c in range(N_CORES):
        band = np.zeros((3, BAND, W0 + 2), np.float32)
        g0 = 640 * c + BAND_OFF
        lo, hi = max(g0, 0), min(g0 + BAND, H)
        band[:, lo - g0: hi - g0, 1: W0 + 1] = xb[:, lo:hi, :]
        w1c = np.concatenate(
            [fc1_w[:, ci * 25600 + 3200 * c: ci * 25600 + 3200 * c + 3200]
             for ci in range(3)], axis=1)     # [1024, 9600]
        m = dict(common)
        m["x"] = band
        mask = np.ones((128, max(N_MASK, 1)), np.float32)
        for i, (_, _, _, entries) in enumerate(MASK_COLS):
            for (p_, which) in entries:
                if (which == 0 and c == 0) or (which == 1 and c == N_CORES - 1):
                    mask[p_, i] = 0.0
        m["mask"] = mask
        m["w1t"] = np.ascontiguousarray(w1c.T).astype(NPBF16)
        in_maps.append(m)
    return in_maps


_NC_CACHE = None

def _get_nc():
    global _NC_CACHE
    if _NC_CACHE is None:
        _NC_CACHE = build_program()
    return _NC_CACHE


def kernel(x, H, W, nTh, nTw,
           w1, w2, w3, w4, w5, w6, w7, w8, w9, w10, w11, w12, w13,
           fc1_w, fc2_w):
    ws = [w1, w2, w3, w4, w5, w6, w7, w8, w9, w10, w11, w12, w13]
    in_maps = _prep_in_maps(x, ws, fc1_w, fc2_w)
    nc = _get_nc()
    res = bass_utils.run_bass_kernel_spmd(nc, in_maps, core_ids=list(range(N_CORES)))
    out = np.zeros((1, 1024), np.float32)
    for c in range(N_CORES):
        out += res.results[c]["q"]
    return out
